# revision 12
# baseline (speedup 1.0000x reference)
"""Causal multi-head attention on 8 TRN2 NeuronCores.

Sharding: data-parallel over batch (2) x tensor-parallel over heads (4 groups
of 4 heads). Core c handles batch c//4, heads [4*(c%4), 4*(c%4)+4).
Each core computes Q/K/V projections for its head slice, causal flash-style
attention, and a partial output projection (Wo row-shard). The host sums the
4 partials per batch and adds bo.

Shapes (per core): X [2048, 1024], WQ/WK/WV [1024, 256], WO [256, 1024].
"""

import numpy as np

import concourse.bass as bass
import concourse.mybir as mybir
import concourse.tile as tile
from concourse import bacc
from concourse.bass_utils import run_bass_kernel_spmd
from concourse.masks import make_identity

B = 2
S = 2048
D = 1024
H_PER_CORE = 4  # heads per core
HD = 64  # head dim
HG = H_PER_CORE * HD  # 256: projection slice width per core
P = 128
NQC = 4  # q chunks of 512
QC = S // NQC  # 512
NKB = S // P  # 16 k-blocks of 128
NEG = -1.0e9

f32 = mybir.dt.float32
f32r = mybir.dt.float32r


def _bc(ap):
    return ap


def build_nc():
    nc = bacc.Bacc()

    X = nc.dram_tensor("X", [S, D], f32r, kind="ExternalInput")
    WQ = nc.dram_tensor("WQ", [D, HG], f32r, kind="ExternalInput")
    WK = nc.dram_tensor("WK", [D, HG], f32r, kind="ExternalInput")
    WV = nc.dram_tensor("WV", [D, HG], f32r, kind="ExternalInput")
    WO = nc.dram_tensor("WO", [HG, D], f32r, kind="ExternalInput")
    BQ = nc.dram_tensor("BQ", [HG], f32, kind="ExternalInput")
    BK = nc.dram_tensor("BK", [HG], f32, kind="ExternalInput")
    BV = nc.dram_tensor("BV", [HG], f32, kind="ExternalInput")
    Y = nc.dram_tensor("Y", [S, D], f32, kind="ExternalOutput")

    X_pt = X.rearrange("(t p) d -> p t d", p=P)  # [128, 16, 1024]
    Y_pt = Y.rearrange("(t p) d -> p t d", p=P)

    with tile.TileContext(nc) as tc:
        with tc.tile_pool(name="persist", bufs=1) as persist:
            # constants
            ident_f32 = persist.tile([P, P], f32)
            make_identity(nc, ident_f32)
            ident = persist.tile([P, P], f32r)
            nc.vector.tensor_copy(ident, ident_f32)
            # diagmask[k, q] = 0 if q >= k else NEG  (added to S^T before exp)
            diagmask = persist.tile([P, P], f32)
            nc.gpsimd.memset(diagmask, 0.0)
            nc.gpsimd.affine_select(
                out=diagmask,
                in_=diagmask,
                compare_op=mybir.AluOpType.is_ge,
                fill=NEG,
                base=0,
                # iota[k, q] = q - k ; keep where >= 0
                pattern=[[1, P]],
                channel_multiplier=-1,
            )

            # biases
            BQs = persist.tile([P, 2], f32)
            nc.sync.dma_start(BQs, BQ.rearrange("(j p) -> p j", p=P))
            BKs = persist.tile([P, 2], f32)
            nc.sync.dma_start(BKs, BK.rearrange("(j p) -> p j", p=P))
            bv1 = persist.tile([1, HG], f32)
            nc.sync.dma_start(bv1, BV[None, :])
            bvb = persist.tile([P, HG], f32)
            nc.gpsimd.partition_broadcast(bvb, bv1[0:1, :])

            # persistent activations
            # QT[pp][p, s] = (X @ WQ + BQ)[s, 128*pp + p]
            QT = [persist.tile([P, S], f32r, name=f"QT{pp}") for pp in range(2)]
            KT = [persist.tile([P, S], f32r, name=f"KT{pp}") for pp in range(2)]
            # V4[p, t, h, d] = (X @ WV + BV)[128*t + p, 64*h + d]; d=64 -> 1.0
            V4 = persist.tile([P, NKB, H_PER_CORE, HD + 1], f32r)
            ones_f32 = persist.tile([P, NKB * H_PER_CORE], f32)
            nc.gpsimd.memset(ones_f32, 1.0)
            nc.vector.tensor_copy(
                V4[:, :, :, HD],
                ones_f32.rearrange("p (t h) -> p t h", t=NKB),
            )
            # ONT[p, j, q] = O_normalized[q, 128*j + p]
            ONT = persist.tile([P, 2, S], f32r)

            # ---- Stage 1: transpose X -> XT;  Stage 2: projections ----
            with (
                tc.tile_pool(name="s12", bufs=2) as p12,
                tc.tile_pool(name="xtp", bufs=1) as xtp,
                tc.tile_pool(name="ps12", bufs=2, space="PSUM") as ps12,
            ):
                XT = xtp.tile([P, D // P, S], f32r)  # [128, 8, 2048]
                for j in range(D // P):
                    xs = p12.tile([P, NKB, P], f32r, tag="xs")
                    nc.sync.dma_start(xs, X_pt[:, :, P * j : P * (j + 1)])
                    for t in range(NKB):
                        tp = ps12.tile([P, P], f32r, tag="tr")
                        nc.tensor.transpose(tp, xs[:, t, :], ident)
                        nc.vector.tensor_copy(XT[:, j, P * t : P * (t + 1)], tp)

                WQs = p12.tile([P, 8, HG], f32r, tag="wq", bufs=1)
                nc.sync.dma_start(WQs, WQ.rearrange("(j p) n -> p j n", p=P))
                WKs = p12.tile([P, 8, HG], f32r, tag="wk", bufs=1)
                nc.sync.dma_start(WKs, WK.rearrange("(j p) n -> p j n", p=P))
                WVs = p12.tile([P, 8, HG], f32r, tag="wv", bufs=1)
                nc.sync.dma_start(WVs, WV.rearrange("(j p) n -> p j n", p=P))

                # V projection: V4[:, t, :, 0:64] = X[t-block] @ WV + BV
                for t in range(NKB):
                    psv = ps12.tile([P, 512], f32, tag="proj")
                    for j in range(8):
                        nc.tensor.matmul(
                            psv[:, :HG],
                            _bc(XT[:, j, P * t : P * (t + 1)]),
                            _bc(WVs[:, j, :]),
                            start=(j == 0),
                            stop=(j == 7),
                        )
                    nc.vector.tensor_tensor(
                        out=V4[:, t, :, 0:HD],
                        in0=psv[:, :HG].rearrange("p (h d) -> p h d", h=H_PER_CORE),
                        in1=bvb.rearrange("p (h d) -> p h d", h=H_PER_CORE),
                        op=mybir.AluOpType.add,
                    )

                # Q^T / K^T projections, head-pair stacked
                for pp in range(2):
                    for nq in range(NQC):
                        sl = slice(QC * nq, QC * (nq + 1))
                        psq = ps12.tile([P, 512], f32, tag="proj")
                        for j in range(8):
                            nc.tensor.matmul(
                                psq,
                                _bc(WQs[:, j, P * pp : P * (pp + 1)]),
                                _bc(XT[:, j, sl]),
                                start=(j == 0),
                                stop=(j == 7),
                            )
                        nc.vector.tensor_scalar_add(
                            QT[pp][:, sl], psq, BQs[:, pp : pp + 1]
                        )
                        psk = ps12.tile([P, 512], f32, tag="proj")
                        for j in range(8):
                            nc.tensor.matmul(
                                psk,
                                _bc(WKs[:, j, P * pp : P * (pp + 1)]),
                                _bc(XT[:, j, sl]),
                                start=(j == 0),
                                stop=(j == 7),
                            )
                        nc.vector.tensor_scalar_add(
                            KT[pp][:, sl], psk, BKs[:, pp : pp + 1]
                        )

            # ---- Stage 3: attention ----
            with (
                tc.tile_pool(name="p3", bufs=2) as p3,
                tc.tile_pool(name="ps3", bufs=1, space="PSUM") as ps3,
            ):
                for pp in range(2):
                    for qc in range(NQC):
                        qb = QC * qc
                        qsl = slice(qb, qb + QC)
                        nkb = 4 * qc + 4  # causal: k-blocks 0..nkb-1
                        ot = [
                            ps3.tile([HD + 1, QC], f32, tag=f"ot{hh}", name=f"ot{hh}_{pp}_{qc}")
                            for hh in range(2)
                        ]
                        pts = []
                        for kg in range((nkb + 1) // 2):
                            kbs = [kb for kb in (2 * kg, 2 * kg + 1) if kb < nkb]
                            st = [
                                ps3.tile(
                                    [P, 2, QC], f32, tag="sc", bufs=3,
                                    name=f"sc{hh}_{pp}_{qc}_{kg}",
                                )
                                for hh in range(2)
                            ]
                            for ii, kb in enumerate(kbs):
                                for hh in range(2):
                                    hsl = slice(HD * hh, HD * (hh + 1))
                                    nc.tensor.matmul(
                                        st[hh][:, ii, :],
                                        _bc(KT[pp][hsl, P * kb : P * (kb + 1)]),
                                        _bc(QT[pp][hsl, qsl]),
                                        start=True,
                                        stop=True,
                                    )
                                    if kb >= 4 * qc:  # diagonal block: causal mask
                                        qloc = P * kb - qb
                                        nc.vector.tensor_tensor(
                                            out=st[hh][:, ii, qloc : qloc + P],
                                            in0=st[hh][:, ii, qloc : qloc + P],
                                            in1=diagmask,
                                            op=mybir.AluOpType.add,
                                        )
                            # exp (scale 1/sqrt(HD) folded in); PT = exp(S^T / 8)
                            pt = [
                                p3.tile(
                                    [P, 2, QC], f32r, tag=f"pt{hh}", bufs=3,
                                    name=f"pt{hh}_{pp}_{qc}_{kg}",
                                )
                                for hh in range(2)
                            ]
                            for hh in range(2):
                                nexp = len(kbs)
                                nc.scalar.activation(
                                    pt[hh][:, :nexp, :],
                                    st[hh][:, :nexp, :],
                                    mybir.ActivationFunctionType.Exp,
                                    bias=0.0,
                                    scale=0.125,
                                )
                            pts.append((kbs, pt))

                        # PV: O^T (+ row sums) accumulated over k-blocks
                        for kbs, pt in pts:
                            for ii, kb in enumerate(kbs):
                                qloc = max(0, P * kb - qb)
                                for hh in range(2):
                                    h = 2 * pp + hh
                                    nc.tensor.matmul(
                                        ot[hh][:, qloc:QC],
                                        _bc(V4[:, kb, h, :]),
                                        _bc(pt[hh][:, ii, qloc:QC]),
                                        start=(kb == 0),
                                        stop=(kb == nkb - 1),
                                    )

                        # normalize: ONT[.., q] = O^T[., q] / l[q]
                        for hh in range(2):
                            l1 = p3.tile(
                                [1, QC], f32, tag=f"l{hh}", bufs=2,
                                name=f"l{hh}_{pp}_{qc}",
                            )
                            nc.scalar.copy(l1, ot[hh][HD : HD + 1, :])
                            r = p3.tile(
                                [HD, QC], f32, tag=f"r{hh}", bufs=2,
                                name=f"r{hh}_{pp}_{qc}",
                            )
                            nc.gpsimd.partition_broadcast(r, l1[0:1, :])
                            nc.vector.reciprocal_approx_fast(r, r)
                            if hh == 0:
                                nc.vector.tensor_tensor(
                                    out=ONT[0:HD, pp, qsl],
                                    in0=ot[hh][0:HD, :],
                                    in1=r,
                                    op=mybir.AluOpType.mult,
                                )
                            else:
                                tmp = p3.tile(
                                    [HD, QC], f32r, tag="nb", bufs=2,
                                    name=f"nb_{pp}_{qc}",
                                )
                                nc.vector.tensor_tensor(
                                    out=tmp,
                                    in0=ot[hh][0:HD, :],
                                    in1=r,
                                    op=mybir.AluOpType.mult,
                                )
                                nc.sync.dma_start(ONT[HD:P, pp, qsl], tmp)

            # ---- Stage 4: output projection (partial Y) ----
            with (
                tc.tile_pool(name="p4", bufs=3) as p4,
                tc.tile_pool(name="ps4", bufs=2, space="PSUM") as ps4,
            ):
                WOs = p4.tile([P, 2, D], f32r, tag="wo", bufs=1)
                nc.sync.dma_start(WOs, WO.rearrange("(j p) n -> p j n", p=P))
                for t in range(NKB):
                    for nn in range(2):
                        psy = ps4.tile([P, 512], f32, tag="yp")
                        for j in range(2):
                            nc.tensor.matmul(
                                psy,
                                _bc(ONT[:, j, P * t : P * (t + 1)]),
                                _bc(WOs[:, j, 512 * nn : 512 * (nn + 1)]),
                                start=(j == 0),
                                stop=(j == 1),
                            )
                        ysb = p4.tile([P, 512], f32, tag="ysb")
                        nc.vector.tensor_copy(ysb, psy)
                        nc.sync.dma_start(
                            Y_pt[:, t, 512 * nn : 512 * (nn + 1)], ysb
                        )

    nc.compile()
    return nc


_NC_CACHE = None


def _get_nc():
    global _NC_CACHE
    if _NC_CACHE is None:
        _NC_CACHE = build_nc()
    return _NC_CACHE


def kernel(X, Wq, bq, Wk, bk, Wv, bv, Wo, bo):
    X = np.asarray(X, np.float32)
    Wq = np.asarray(Wq, np.float32)
    Wk = np.asarray(Wk, np.float32)
    Wv = np.asarray(Wv, np.float32)
    Wo = np.asarray(Wo, np.float32)
    bq = np.asarray(bq, np.float32)
    bk = np.asarray(bk, np.float32)
    bv = np.asarray(bv, np.float32)
    bo = np.asarray(bo, np.float32)

    nc = _get_nc()
    in_maps = []
    for c in range(8):
        b, hg = c // 4, c % 4
        sl = slice(HG * hg, HG * (hg + 1))
        in_maps.append(
            {
                "X": np.ascontiguousarray(X[b]),
                "WQ": np.ascontiguousarray(Wq[:, sl]),
                "WK": np.ascontiguousarray(Wk[:, sl]),
                "WV": np.ascontiguousarray(Wv[:, sl]),
                "WO": np.ascontiguousarray(Wo[sl, :]),
                "BQ": np.ascontiguousarray(bq[sl]),
                "BK": np.ascontiguousarray(bk[sl]),
                "BV": np.ascontiguousarray(bv[sl]),
            }
        )
    res = run_bass_kernel_spmd(nc, in_maps, core_ids=list(range(8)))
    ys = [r["Y"] for r in res.results]
    out = np.stack(
        [ys[0] + ys[1] + ys[2] + ys[3], ys[4] + ys[5] + ys[6] + ys[7]]
    )
    return (out + bo).astype(np.float32)


# revision 14
# speedup vs baseline: 1.1631x; 1.1631x over previous
"""Causal multi-head attention on 8 TRN2 NeuronCores.

Sharding: data-parallel over batch (2) x tensor-parallel over heads (4 groups
of 4 heads). Core c handles batch c//4, heads [4*(c%4), 4*(c%4)+4).
Each core computes Q/K/V projections for its head slice, causal flash-style
attention, and a partial output projection (Wo row-shard). The host sums the
4 partials per batch and adds bo.

Shapes (per core): X [2048, 1024], WQ/WK/WV [1024, 256], WO [256, 1024].
"""

import numpy as np

import concourse.bass as bass
import concourse.mybir as mybir
import concourse.tile as tile
from concourse import bacc
from concourse.bass_utils import run_bass_kernel_spmd
from concourse.masks import make_identity

B = 2
S = 2048
D = 1024
H_PER_CORE = 4  # heads per core
HD = 64  # head dim
HG = H_PER_CORE * HD  # 256: projection slice width per core
P = 128
NQC = 4  # q chunks of 512
QC = S // NQC  # 512
NKB = S // P  # 16 k-blocks of 128
NEG = -1.0e9

f32 = mybir.dt.float32
f32r = mybir.dt.float32r


def build_nc():
    nc = bacc.Bacc()

    X = nc.dram_tensor("X", [S, D], f32r, kind="ExternalInput")
    WQ = nc.dram_tensor("WQ", [D, HG], f32r, kind="ExternalInput")
    WK = nc.dram_tensor("WK", [D, HG], f32r, kind="ExternalInput")
    WV = nc.dram_tensor("WV", [D, HG], f32r, kind="ExternalInput")
    WO = nc.dram_tensor("WO", [HG, D], f32r, kind="ExternalInput")
    BQ = nc.dram_tensor("BQ", [HG], f32, kind="ExternalInput")
    BK = nc.dram_tensor("BK", [HG], f32, kind="ExternalInput")
    BV = nc.dram_tensor("BV", [HG], f32, kind="ExternalInput")
    Y = nc.dram_tensor("Y", [S, D], f32, kind="ExternalOutput")

    X_pt = X.rearrange("(t p) d -> p t d", p=P)  # [128, 16, 1024]
    Y_pt = Y.rearrange("(t p) d -> p t d", p=P)

    with tile.TileContext(nc) as tc:
        with (
            tc.tile_pool(name="persist", bufs=1) as persist,
            tc.tile_pool(name="sb", bufs=2) as sb,
            tc.tile_pool(name="ps", bufs=1, space="PSUM") as ps,
        ):
            # ---- constants ----
            ident_f32 = persist.tile([P, P], f32)
            make_identity(nc, ident_f32)
            ident = persist.tile([P, P], f32r)
            nc.vector.tensor_copy(ident, ident_f32)
            # diagmask[k, q] = 0 if q >= k else NEG  (added to S^T before exp)
            diagmask = persist.tile([P, P], f32)
            nc.gpsimd.memset(diagmask, 0.0)
            nc.gpsimd.affine_select(
                out=diagmask,
                in_=diagmask,
                compare_op=mybir.AluOpType.is_ge,
                fill=NEG,
                base=0,
                pattern=[[1, P]],  # iota[k, q] = q - k ; keep where >= 0
                channel_multiplier=-1,
            )

            # ---- biases ----
            BQs = persist.tile([P, 2], f32)
            nc.sync.dma_start(BQs, BQ.rearrange("(j p) -> p j", p=P))
            BKs = persist.tile([P, 2], f32)
            nc.sync.dma_start(BKs, BK.rearrange("(j p) -> p j", p=P))
            bv1 = persist.tile([1, HG], f32)
            nc.sync.dma_start(bv1, BV[None, :])
            bvb = persist.tile([P, HG], f32)
            nc.gpsimd.partition_broadcast(bvb, bv1[0:1, :])

            # ---- persistent activations ----
            QT = [persist.tile([P, S], f32r, name=f"QT{pp}") for pp in range(2)]
            KT = [persist.tile([P, S], f32r, name=f"KT{pp}") for pp in range(2)]
            # V4[p, t, h, d] = (X @ WV + BV)[128*t + p, 64*h + d]; d=64 -> 1.0
            V4 = persist.tile([P, NKB, H_PER_CORE, HD + 1], f32r)
            ones_f32 = persist.tile([P, NKB * H_PER_CORE], f32)
            nc.gpsimd.memset(ones_f32, 1.0)
            nc.vector.tensor_copy(
                V4[:, :, :, HD], ones_f32.rearrange("p (t h) -> p t h", t=NKB)
            )
            # ONT[p, j, q] = O_normalized[q, 128*j + p]
            ONT = persist.tile([P, 2, S], f32r)
            XT = persist.tile([P, D // P, S], f32r)  # [128, 8, 2048]

            # weights
            WQs = persist.tile([P, 8, HG], f32r)
            nc.sync.dma_start(WQs, WQ.rearrange("(j p) n -> p j n", p=P))
            WKs = persist.tile([P, 8, HG], f32r)
            nc.sync.dma_start(WKs, WK.rearrange("(j p) n -> p j n", p=P))
            WVs = persist.tile([P, 8, HG], f32r)
            nc.sync.dma_start(WVs, WV.rearrange("(j p) n -> p j n", p=P))
            WOs = persist.tile([P, 2, D], f32r)
            nc.sync.dma_start(WOs, WO.rearrange("(j p) n -> p j n", p=P))

            # ---- Stage 1: transpose X -> XT ----
            for j in range(D // P):
                xs = sb.tile([P, NKB // 2, P], f32r, tag="xs", name=f"xs{j}")
                nc.sync.dma_start(
                    xs, X_pt[:, 0 : NKB // 2, P * j : P * (j + 1)]
                )
                xs2 = sb.tile([P, NKB // 2, P], f32r, tag="xs", name=f"xs2_{j}")
                nc.sync.dma_start(
                    xs2, X_pt[:, NKB // 2 : NKB, P * j : P * (j + 1)]
                )
                for t in range(NKB):
                    tp = ps.tile(
                        [P, 512], f32r, tag="proj", bufs=2, name=f"tr{j}_{t}"
                    )
                    xsrc = xs if t < NKB // 2 else xs2
                    nc.tensor.transpose(tp[:, :P], xsrc[:, t % (NKB // 2), :], ident)
                    nc.any.tensor_copy(XT[:, j, P * t : P * (t + 1)], tp[:, :P])

            # ---- Stage 2a: V projection ----
            for t in range(NKB):
                psv = ps.tile([P, 512], f32, tag="proj", bufs=2, name=f"psv{t}")
                for j in range(8):
                    nc.tensor.matmul(
                        psv[:, :HG],
                        XT[:, j, P * t : P * (t + 1)],
                        WVs[:, j, :],
                        start=(j == 0),
                        stop=(j == 7),
                    )
                nc.vector.tensor_tensor(
                    out=V4[:, t, :, 0:HD],
                    in0=psv[:, :HG].rearrange("p (h d) -> p h d", h=H_PER_CORE),
                    in1=bvb.rearrange("p (h d) -> p h d", h=H_PER_CORE),
                    op=mybir.AluOpType.add,
                )

            def emit_qk_proj(pp, nq):
                sl = slice(QC * nq, QC * (nq + 1))
                psq = ps.tile(
                    [P, 512], f32, tag="proj", bufs=2, name=f"psq{pp}_{nq}"
                )
                for j in range(8):
                    nc.tensor.matmul(
                        psq,
                        WQs[:, j, P * pp : P * (pp + 1)],
                        XT[:, j, sl],
                        start=(j == 0),
                        stop=(j == 7),
                    )
                nc.vector.tensor_scalar_add(QT[pp][:, sl], psq, BQs[:, pp : pp + 1])
                psk = ps.tile(
                    [P, 512], f32, tag="proj", bufs=2, name=f"psk{pp}_{nq}"
                )
                for j in range(8):
                    nc.tensor.matmul(
                        psk,
                        WKs[:, j, P * pp : P * (pp + 1)],
                        XT[:, j, sl],
                        start=(j == 0),
                        stop=(j == 7),
                    )
                nc.vector.tensor_scalar_add(KT[pp][:, sl], psk, BKs[:, pp : pp + 1])

            def emit_pv(pp, qc, ot, kbs, pt):
                qb = QC * qc
                nkb = 4 * qc + 4
                for ii, kb in enumerate(kbs):
                    qloc = max(0, P * kb - qb)
                    for hh in range(2):
                        h = 2 * pp + hh
                        nc.tensor.matmul(
                            ot[hh][:, qloc:QC],
                            V4[:, kb, h, :],
                            pt[hh][:, ii, qloc:QC],
                            start=(kb == 0),
                            stop=(kb == nkb - 1),
                        )

            def emit_attention(pp, qc):
                qb = QC * qc
                qsl = slice(qb, qb + QC)
                nkb = 4 * qc + 4  # causal: k-blocks 0..nkb-1
                ot = [
                    ps.tile(
                        [HD + 1, QC], f32, tag=f"ot{hh}", bufs=1,
                        name=f"ot{hh}_{pp}_{qc}",
                    )
                    for hh in range(2)
                ]
                prev = None
                for kg in range((nkb + 1) // 2):
                    kbs = [kb for kb in (2 * kg, 2 * kg + 1) if kb < nkb]
                    st = [
                        ps.tile(
                            [P, 2, QC], f32, tag="sc", bufs=2,
                            name=f"sc{hh}_{pp}_{qc}_{kg}",
                        )
                        for hh in range(2)
                    ]
                    for ii, kb in enumerate(kbs):
                        for hh in range(2):
                            hsl = slice(HD * hh, HD * (hh + 1))
                            nc.tensor.matmul(
                                st[hh][:, ii, :],
                                KT[pp][hsl, P * kb : P * (kb + 1)],
                                QT[pp][hsl, qsl],
                                start=True,
                                stop=True,
                            )
                            if kb >= 4 * qc:  # diagonal block: causal mask
                                qloc = P * kb - qb
                                nc.vector.tensor_tensor(
                                    out=st[hh][:, ii, qloc : qloc + P],
                                    in0=st[hh][:, ii, qloc : qloc + P],
                                    in1=diagmask,
                                    op=mybir.AluOpType.add,
                                )
                    pt = [
                        sb.tile(
                            [P, 2, QC], f32r, tag=f"pt{hh}", bufs=2,
                            name=f"pt{hh}_{pp}_{qc}_{kg}",
                        )
                        for hh in range(2)
                    ]
                    for hh in range(2):
                        nexp = len(kbs)
                        nc.scalar.activation(
                            pt[hh][:, :nexp, :],
                            st[hh][:, :nexp, :],
                            mybir.ActivationFunctionType.Exp,
                            bias=0.0,
                            scale=0.125,
                        )
                    if prev is not None:
                        emit_pv(pp, qc, ot, *prev)
                    prev = (kbs, pt)
                emit_pv(pp, qc, ot, *prev)

                # normalize: ONT[.., q] = O^T[., q] / l[q]
                for hh in range(2):
                    l1 = sb.tile(
                        [1, QC], f32, tag="l", bufs=2, name=f"l{hh}_{pp}_{qc}"
                    )
                    nc.vector.tensor_copy(l1, ot[hh][HD : HD + 1, :])
                    r = sb.tile(
                        [HD, QC], f32, tag="rr", bufs=2, name=f"r{hh}_{pp}_{qc}"
                    )
                    nc.gpsimd.partition_broadcast(r, l1[0:1, :])
                    nc.vector.reciprocal_approx_fast(r, r)
                    if hh == 0:
                        nc.vector.tensor_tensor(
                            out=ONT[0:HD, pp, qsl],
                            in0=ot[hh][0:HD, :],
                            in1=r,
                            op=mybir.AluOpType.mult,
                        )
                    else:
                        tmp = sb.tile(
                            [HD, QC], f32r, tag="nb", bufs=2, name=f"nb_{pp}_{qc}"
                        )
                        nc.vector.tensor_tensor(
                            out=tmp,
                            in0=ot[hh][0:HD, :],
                            in1=r,
                            op=mybir.AluOpType.mult,
                        )
                        nc.sync.dma_start(ONT[HD:P, pp, qsl], tmp)

            def emit_oproj(qc):
                # output projection for q-blocks of this q-chunk
                for t in range(4 * qc, 4 * qc + 4):
                    for nn in range(2):
                        psy = ps.tile(
                            [P, 512], f32, tag="proj", bufs=2,
                            name=f"psy{t}_{nn}",
                        )
                        for j in range(2):
                            nc.tensor.matmul(
                                psy,
                                ONT[:, j, P * t : P * (t + 1)],
                                WOs[:, j, 512 * nn : 512 * (nn + 1)],
                                start=(j == 0),
                                stop=(j == 1),
                            )
                        ysb = sb.tile(
                            [P, 512], f32, tag="ysb", bufs=2, name=f"ysb{t}_{nn}"
                        )
                        nc.any.tensor_copy(ysb, psy)
                        nc.sync.dma_start(
                            Y_pt[:, t, 512 * nn : 512 * (nn + 1)], ysb
                        )

            # ---- Stage 2b/3/4 interleaved ----
            for nq in range(NQC):
                emit_qk_proj(0, nq)
            for qc in range(NQC):
                emit_attention(0, qc)
                emit_qk_proj(1, qc)  # PE filler under pair-0 softmax
            for qc in range(NQC):
                emit_attention(1, qc)
                emit_oproj(qc)  # PE filler under pair-1 softmax

    nc.compile()
    return nc


_NC_CACHE = None


def _get_nc():
    global _NC_CACHE
    if _NC_CACHE is None:
        _NC_CACHE = build_nc()
    return _NC_CACHE


def kernel(X, Wq, bq, Wk, bk, Wv, bv, Wo, bo):
    X = np.asarray(X, np.float32)
    Wq = np.asarray(Wq, np.float32)
    Wk = np.asarray(Wk, np.float32)
    Wv = np.asarray(Wv, np.float32)
    Wo = np.asarray(Wo, np.float32)
    bq = np.asarray(bq, np.float32)
    bk = np.asarray(bk, np.float32)
    bv = np.asarray(bv, np.float32)
    bo = np.asarray(bo, np.float32)

    nc = _get_nc()
    in_maps = []
    for c in range(8):
        b, hg = c // 4, c % 4
        sl = slice(HG * hg, HG * (hg + 1))
        in_maps.append(
            {
                "X": np.ascontiguousarray(X[b]),
                "WQ": np.ascontiguousarray(Wq[:, sl]),
                "WK": np.ascontiguousarray(Wk[:, sl]),
                "WV": np.ascontiguousarray(Wv[:, sl]),
                "WO": np.ascontiguousarray(Wo[sl, :]),
                "BQ": np.ascontiguousarray(bq[sl]),
                "BK": np.ascontiguousarray(bk[sl]),
                "BV": np.ascontiguousarray(bv[sl]),
            }
        )
    res = run_bass_kernel_spmd(nc, in_maps, core_ids=list(range(8)))
    ys = [r["Y"] for r in res.results]
    out = np.stack(
        [ys[0] + ys[1] + ys[2] + ys[3], ys[4] + ys[5] + ys[6] + ys[7]]
    )
    return (out + bo).astype(np.float32)
